# revision 1
# baseline (speedup 1.0000x reference)
"""CenterNet loss (heatmap focal + giou regression) on 8 Trainium2 cores.

Data-parallel over the M (pixel) axis: each core gets M/8 rows of every
M-sized tensor, positives are routed to the shard that owns their row, and
the three scalar loss sums are combined on the host.

Math (per core, all engines fed from one streaming pass over the shard):
  neg:  sum softplus(x) * sigmoid(x)^2 * (1-hm)^4
        with softplus(x) = ln(1+e^x) = -log(1-sigmoid(x))
        and  sigmoid(x)^2 = exp(2*(x - softplus(x)))
        (only Exp/Ln/square ACT funcs -> single activation table set)
  pos:  sum softplus(-x_g) * exp(-2*(x_g + softplus(-x_g))) * mask
        where x_g are the positives' logits, gathered on-device via
        indirect DMA; equals -log(p)*(1-p)^2 summed over real positives.
  reg:  A = sum (giou_pen - iou) * w ; B = sum w   (loss = B + A)
Host:  pos_loss = POS_W*ALPHA*S_pos/2048
       neg_loss = NEG_W*(1-ALPHA)*S_neg/2048
       reg_loss = REG_W*(A+B)/max(B,1)
"""

import numpy as np

M_TOTAL = 349184
C = 80
N_CORES = 8
R = M_TOTAL // N_CORES        # 43648 rows per core
P = 128
NF = R * C                    # 3,491,840 flat f32 per heatmap shard
FN = NF // P                  # 27,280 free elems per partition
FR = R * 4 // P               # 1,364 reg elems per partition
BOXES = FR // 4               # 341 boxes per partition
POS_CAP = 512                 # padded positives per core
PC = POS_CAP // P             # 4 offset columns
N_POS = 2048

ALPHA = 0.25
POS_W = 1.0
NEG_W = 1.0
REG_W = 2.0

# streaming tile sizes along the free dim (sum == FN)
F_TILES = [1024, 1024] + [2048] * 11 + [1024, 1024, 656]
# chain pool buffer counts (xp, hp, ep, ap, s4p, tsp, t2p)
BUFS = dict(xp=4, hp=3, ep=2, ap=3, s4p=2, tsp=2, t2p=2, s4b=2)
DEBUG_SKIP = set()   # subset of {"reg", "pos", "mm"} to disable parts
NO_PE = True         # per-iter DVE reduce (PE/PSUM path measured slower on HW)
REPEAT = 1           # timing aid: stream the shard REPEAT times in one program

TRACE = False          # set True (e.g. from test.py) to capture an NTFF profile
LAST_RESULTS = None    # BassKernelResults of the most recent run

_CACHED_NC = None


def _build_nc():
    import concourse.bass as bass
    import concourse.bacc as bacc
    import concourse.tile as tile
    from concourse import mybir

    f32 = mybir.dt.float32
    bf16 = mybir.dt.bfloat16
    i32 = mybir.dt.int32
    Alu = mybir.AluOpType
    Act = mybir.ActivationFunctionType

    nc = bacc.Bacc(trn_type="TRN2")

    lg = nc.declare_dram_parameter("lg", [NF, 1], f32, isOutput=False)
    hm = nc.declare_dram_parameter("hm", [NF, 1], f32, isOutput=False)
    rp = nc.declare_dram_parameter("rp", [P, FR], f32, isOutput=False)
    rt = nc.declare_dram_parameter("rt", [P, FR], f32, isOutput=False)
    pval = nc.declare_dram_parameter("pval", [P, PC], f32, isOutput=False)
    pmask = nc.declare_dram_parameter("pmask", [P, PC], f32, isOutput=False)
    out = nc.declare_dram_parameter("out", [P, 4], f32, isOutput=True)

    lgv = lg.rearrange("(p n) o -> p (n o)", p=P)   # [128, FN]
    hmv = hm.rearrange("(p n) o -> p (n o)", p=P)

    with tile.TileContext(nc) as tc:
        with (
            tc.tile_pool(name="xp", bufs=BUFS["xp"]) as xp,
            tc.tile_pool(name="hp", bufs=BUFS["hp"]) as hp,
            tc.tile_pool(name="ep", bufs=BUFS["ep"]) as ep,
            tc.tile_pool(name="ap", bufs=BUFS["ap"]) as ap,
            tc.tile_pool(name="s4p", bufs=BUFS["s4p"]) as s4p,
            tc.tile_pool(name="tsp", bufs=BUFS["tsp"]) as tsp,
            tc.tile_pool(name="s4b", bufs=BUFS["s4b"]) as s4b,
            tc.tile_pool(name="t2p", bufs=BUFS["t2p"]) as t2p,
            tc.tile_pool(name="small", bufs=1) as small,
            tc.tile_pool(name="regp", bufs=1) as regp,
            tc.tile_pool(name="rs", bufs=1) as rs,
            tc.tile_pool(name="pp", bufs=1, space="PSUM") as pp,
        ):
            acc_pos = small.tile([P, 1], f32)
            acc_rega = small.tile([P, 1], f32)
            acc_w = small.tile([P, 1], f32)
            if "pos" in DEBUG_SKIP:
                nc.vector.memset(acc_pos[:], 0.0)
            if "reg" in DEBUG_SKIP:
                nc.vector.memset(acc_rega[:], 0.0)
                nc.vector.memset(acc_w[:], 0.0)

            # ---------------- negatives: streamed focal-neg ---------------
            # Tile lifetimes are kept short (no cross-stage in-place reuse)
            # so DMA/ACT/DVE/Pool pipeline across iterations. The final
            # product is written bf16 and reduced on the (otherwise idle)
            # tensor engine via a ones-matmul accumulating in PSUM.
            if not NO_PE:
                ones = small.tile([P, 1], bf16)
                nc.vector.memset(ones[:], 1.0)
                psum_acc = pp.tile([1, 512], f32)
            parts = small.tile([P, len(F_TILES) * REPEAT], f32)
            mm_state = dict(n=0)
            total_mm = sum(len(range(0, F, 512)) for F in F_TILES * REPEAT)

            def pos_gen():
                # ---------------- positives: indirect gather + focal-pos ------
                mskt = small.tile([P, PC], f32)
                nc.sync.dma_start(out=mskt[:], in_=pmask[:])
                yield
                xg = small.tile([P, PC], f32)
                nc.sync.dma_start(out=xg[:], in_=pval[:])
                yield
                e2 = small.tile([P, PC], f32)
                nc.scalar.activation(e2[:], xg[:], Act.Exp, scale=-1.0)   # e^-x
                yield
                sp2 = small.tile([P, PC], f32)
                nc.scalar.activation(sp2[:], e2[:], Act.Ln, bias=1.0)     # softplus(-x)
                yield
                a2 = small.tile([P, PC], f32)
                nc.vector.tensor_tensor(out=a2[:], in0=xg[:], in1=sp2[:], op=Alu.add)
                yield
                nc.scalar.activation(a2[:], a2[:], Act.Exp, scale=-2.0)   # (1-p)^2
                yield
                nc.vector.tensor_tensor(out=e2[:], in0=sp2[:], in1=a2[:], op=Alu.mult)
                yield
                nc.vector.tensor_tensor(out=e2[:], in0=e2[:], in1=mskt[:], op=Alu.mult)
                yield
                nc.vector.tensor_reduce(out=acc_pos[:], in_=e2[:],
                                        axis=mybir.AxisListType.X, op=Alu.add)
                yield
            if "pos" not in DEBUG_SKIP:
                for _ in pos_gen():
                    pass
            def reg_gen():
                # regression (giou): batched component ops, split DVE/Pool
                rpt = regp.tile([P, FR], f32)
                nc.sync.dma_start(out=rpt[:], in_=rp[:])
                rtt = regp.tile([P, FR], f32)
                nc.sync.dma_start(out=rtt[:], in_=rt[:])
                yield
                pv = rpt[:].rearrange("p (n c) -> p n c", c=4)
                tv = rtt[:].rearrange("p (n c) -> p n c", c=4)

                def T(name, shape=None):
                    return rs.tile(shape or [P, BOXES], f32, name=name, tag=name)

                def eng():
                    return nc.gpsimd

                mm1 = T("mm1", [P, BOXES, 2])
                nc.vector.tensor_tensor(out=mm1[:], in0=tv[:, :, 0:2], in1=tv[:, :, 2:4], op=Alu.max)
                yield
                mx = T("mx")
                nc.vector.tensor_tensor(out=mx[:], in0=mm1[:, :, 0], in1=mm1[:, :, 1], op=Alu.max)
                yield
                w = T("w")
                nc.vector.tensor_scalar(out=w[:], in0=mx[:], scalar1=0.0, scalar2=None,
                                        op0=Alu.is_ge)
                yield
                wu = rs.tile([P, BOXES], mybir.dt.uint8, name="wu", tag="wu")
                nc.vector.tensor_scalar(out=wu[:], in0=mx[:], scalar1=0.0, scalar2=None,
                                        op0=Alu.is_ge)
                yield
                safe = regp.tile([P, FR], f32)
                nc.vector.memset(safe[:], 1.0)
                yield
                sv = safe[:].rearrange("p (n c) -> p n c", c=4)
                wb = bass.AP(tensor=wu[:].tensor, offset=wu[:].offset,
                             ap=list(wu[:].ap) + [[0, 2]])
                nc.vector.copy_predicated(out=sv[:, :, 0:2], mask=wb, data=tv[:, :, 0:2])
                yield
                nc.vector.copy_predicated(out=sv[:, :, 2:4], mask=wb, data=tv[:, :, 2:4])
                yield
                sp2 = T("sp2", [P, BOXES, 2])
                eng().tensor_tensor(out=sp2[:], in0=pv[:, :, 0:2], in1=pv[:, :, 2:4], op=Alu.add)
                yield
                st2 = T("st2", [P, BOXES, 2])
                eng().tensor_tensor(out=st2[:], in0=sv[:, :, 0:2], in1=sv[:, :, 2:4], op=Alu.add)
                yield
                pa = T("pa")
                eng().tensor_tensor(out=pa[:], in0=sp2[:, :, 0], in1=sp2[:, :, 1], op=Alu.mult)
                yield
                ta = T("ta")
                eng().tensor_tensor(out=ta[:], in0=st2[:, :, 0], in1=st2[:, :, 1], op=Alu.mult)
                yield
                mn = regp.tile([P, FR], f32, name="mn", tag="mn")
                nc.vector.tensor_tensor(out=mn[:], in0=rpt[:], in1=safe[:], op=Alu.min)
                yield
                mx2 = regp.tile([P, FR], f32, name="mx2", tag="mx2")
                nc.vector.tensor_tensor(out=mx2[:], in0=rpt[:], in1=safe[:], op=Alu.max)
                yield
                mnv = mn[:].rearrange("p (n c) -> p n c", c=4)
                mxv = mx2[:].rearrange("p (n c) -> p n c", c=4)
                wi = T("wi")
                eng().tensor_tensor(out=wi[:], in0=mnv[:, :, 0], in1=mnv[:, :, 2], op=Alu.add)
                yield
                hi = T("hi")
                eng().tensor_tensor(out=hi[:], in0=mnv[:, :, 1], in1=mnv[:, :, 3], op=Alu.add)
                yield
                gw = T("gw")
                eng().tensor_tensor(out=gw[:], in0=mxv[:, :, 0], in1=mxv[:, :, 2], op=Alu.add)
                yield
                gh = T("gh")
                eng().tensor_tensor(out=gh[:], in0=mxv[:, :, 1], in1=mxv[:, :, 3], op=Alu.add)
                yield
                ac = T("ac")
                eng().tensor_tensor(out=ac[:], in0=gw[:], in1=gh[:], op=Alu.mult)
                yield
                ai = T("ai")
                eng().tensor_tensor(out=ai[:], in0=wi[:], in1=hi[:], op=Alu.mult)
                yield
                au = T("au")
                eng().tensor_tensor(out=au[:], in0=ta[:], in1=pa[:], op=Alu.add)
                yield
                eng().tensor_tensor(out=au[:], in0=au[:], in1=ai[:], op=Alu.subtract)
                yield
                eng().tensor_scalar(out=ai[:], in0=ai[:], scalar1=1.0, scalar2=None,
                                    op0=Alu.add)
                yield
                iou = T("iou")
                nc.vector.tensor_scalar(out=iou[:], in0=au[:], scalar1=1.0, scalar2=None,
                                        op0=Alu.add)
                nc.vector.reciprocal(out=iou[:], in_=iou[:])
                nc.vector.tensor_tensor(out=iou[:], in0=ai[:], in1=iou[:], op=Alu.mult)
                yield
                nm = T("nm")
                eng().tensor_tensor(out=nm[:], in0=ac[:], in1=au[:], op=Alu.subtract)
                yield
                eng().tensor_scalar(out=ac[:], in0=ac[:], scalar1=1e-7, scalar2=None,
                                    op0=Alu.add)
                yield
                nc.vector.reciprocal(out=ac[:], in_=ac[:])
                nc.vector.tensor_tensor(out=nm[:], in0=nm[:], in1=ac[:], op=Alu.mult)
                yield
                nc.vector.tensor_tensor(out=nm[:], in0=nm[:], in1=iou[:], op=Alu.subtract)
                yield
                nc.vector.tensor_tensor(out=nm[:], in0=nm[:], in1=w[:], op=Alu.mult)
                yield
                nc.vector.tensor_reduce(out=acc_rega[:], in_=nm[:],
                                        axis=mybir.AxisListType.X, op=Alu.add)
                yield
                nc.vector.tensor_reduce(out=acc_w[:], in_=w[:],
                                        axis=mybir.AxisListType.X, op=Alu.add)
                yield
            _rg = reg_gen() if "reg" not in DEBUG_SKIP else iter(())

            # software-pipelined emission: round r emits loads for iter r,
            # head compute (e/sp/s4/a/ts4) for iter r-1, and tail compute
            # (p2/t2/matmul) for iter r-2, so each engine's in-order
            # instruction stream interleaves adjacent iterations.
            ft = F_TILES * REPEAT
            n_it = len(ft)
            offs = [sum(ft[:i]) % FN for i in range(n_it)]
            st = [dict() for _ in range(n_it)]

            def emit_load(i):
                F = ft[i]
                o = offs[i]
                xt = xp.tile([P, F], f32, tag="xt", name=f"xt{i}")
                nc.sync.dma_start(out=xt[:], in_=lgv[:, o:o + F])
                ht = hp.tile([P, F], f32, tag="ht", name=f"ht{i}")
                nc.sync.dma_start(out=ht[:], in_=hmv[:, o:o + F])
                st[i].update(xt=xt, ht=ht)

            def emit_head(i):
                F = ft[i]
                xt, ht = st[i]["xt"], st[i]["ht"]
                e = ep.tile([P, F], f32, tag="e", name=f"e{i}")
                nc.scalar.activation(e[:], xt[:], Act.Exp)            # e^x
                nc.scalar.activation(e[:], e[:], Act.Ln, bias=1.0)    # sp=softplus
                s2 = s4p.tile([P, F], f32, tag="s2", name=f"s2{i}")
                nc.scalar.activation(s2[:], ht[:], Act.Square, scale=-1.0,
                                     bias=1.0)                        # (1-hm)^2
                at = ap.tile([P, F], f32, tag="at", name=f"at{i}")
                nc.gpsimd.tensor_tensor(out=at[:], in0=xt[:], in1=e[:], op=Alu.subtract)
                nc.scalar.activation(at[:], at[:], Act.Exp, scale=2.0)  # p^2
                s4 = s4b.tile([P, F], f32, tag="s4", name=f"s4{i}")
                nc.vector.tensor_tensor(out=s4[:], in0=s2[:], in1=s2[:], op=Alu.mult)
                st[i].update(at=at, e=e, s4=s4)

            def emit_tail(i):
                F = ft[i]
                at, e = st[i]["at"], st[i]["e"]
                s4 = st[i]["s4"]
                t = t2p.tile([P, F], f32, tag="t", name=f"t{i}")
                nc.gpsimd.tensor_tensor(out=t[:], in0=e[:], in1=at[:], op=Alu.mult)
                if NO_PE:
                    nc.vector.tensor_tensor(out=t[:], in0=t[:], in1=s4[:], op=Alu.mult)
                    nc.vector.tensor_reduce(out=parts[:, i:i + 1], in_=t[:],
                                            axis=mybir.AxisListType.X, op=Alu.add)
                else:
                    t2 = tsp.tile([P, F], bf16, tag="t2", name=f"t2{i}")
                    nc.vector.tensor_tensor(out=t2[:], in0=t[:], in1=s4[:], op=Alu.mult)
                    for s in range(0, F, 512):
                        sl = min(512, F - s)
                        nc.tensor.matmul(
                            out=psum_acc[:, :sl],
                            lhsT=ones[:],
                            rhs=t2[:, s:s + sl],
                            start=(mm_state["n"] == 0),
                            stop=(mm_state["n"] == total_mm - 1),
                        )
                        mm_state["n"] += 1

            n_rounds = n_it + 2
            for r in range(n_rounds):
                if r < n_it:
                    emit_load(r)
                if 0 <= r - 1 < n_it:
                    emit_head(r - 1)
                if 0 <= r - 2 < n_it:
                    emit_tail(r - 2)
                if r >= 2:
                    for _ in range(6):
                        if next(_rg, "done") == "done":
                            break
            for _ in _rg:
                pass

            accs = small.tile([P, 4], f32)
            nc.vector.memset(accs[:], 0.0)
            if NO_PE:
                nc.vector.tensor_reduce(out=accs[:, 0:1], in_=parts[:, :],
                                        axis=mybir.AxisListType.X, op=Alu.add)
            else:
                nc.vector.tensor_reduce(out=accs[:1, 0:1], in_=psum_acc[:, :],
                                        axis=mybir.AxisListType.X, op=Alu.add)
            if NO_PE:
                pass
            nc.vector.tensor_copy(out=accs[:, 1:2], in_=acc_pos[:])
            nc.vector.tensor_copy(out=accs[:, 2:3], in_=acc_rega[:])
            nc.vector.tensor_copy(out=accs[:, 3:4], in_=acc_w[:])
            nc.sync.dma_start(out=out[:], in_=accs[:])

    nc.finalize()
    return nc


def _get_nc():
    global _CACHED_NC
    if _CACHED_NC is None:
        _CACHED_NC = _build_nc()
    return _CACHED_NC


def _shard_inputs(logits_pred, flattened_hms, reg_pred, reg_targets,
                  pos_inds, labels):
    logits_pred = np.ascontiguousarray(logits_pred, dtype=np.float32)
    flattened_hms = np.ascontiguousarray(flattened_hms, dtype=np.float32)
    reg_pred = np.ascontiguousarray(reg_pred, dtype=np.float32)
    reg_targets = np.ascontiguousarray(reg_targets, dtype=np.float32)
    pos_inds = np.asarray(pos_inds).astype(np.int64)
    labels = np.asarray(labels).astype(np.int64)

    in_maps = []
    for s in range(N_CORES):
        r0, r1 = s * R, (s + 1) * R
        sel = np.nonzero((pos_inds >= r0) & (pos_inds < r1))[0]
        cnt = sel.size
        assert cnt <= POS_CAP, f"shard {s} has {cnt} positives > {POS_CAP}"
        vals = np.zeros(POS_CAP, np.float32)
        msk = np.zeros(POS_CAP, np.float32)
        vals[:cnt] = logits_pred[pos_inds[sel], labels[sel]]
        msk[:cnt] = 1.0
        in_maps.append({
            "lg": logits_pred[r0:r1].reshape(NF, 1),
            "hm": flattened_hms[r0:r1].reshape(NF, 1),
            "rp": reg_pred[r0:r1].reshape(P, FR),
            "rt": reg_targets[r0:r1].reshape(P, FR),
            "pval": vals.reshape(P, PC),
            "pmask": msk.reshape(P, PC),
        })
    return in_maps


def kernel(logits_pred, flattened_hms, reg_pred, reg_targets,
           pos_inds, labels):
    global LAST_RESULTS
    from concourse.bass_utils import run_bass_kernel_spmd

    nc = _get_nc()
    in_maps = _shard_inputs(logits_pred, flattened_hms, reg_pred, reg_targets,
                            pos_inds, labels)
    res = run_bass_kernel_spmd(nc, in_maps, list(range(N_CORES)), trace=TRACE)
    LAST_RESULTS = res

    parts = np.zeros(4, np.float64)
    for s in range(N_CORES):
        parts += res.results[s]["out"].astype(np.float64).sum(axis=0)
    s_neg, s_pos, a_reg, b_w = parts

    pos_loss = POS_W * ALPHA * s_pos / N_POS
    neg_loss = NEG_W * (1.0 - ALPHA) * s_neg / N_POS
    reg_loss = REG_W * (a_reg + b_w) / max(b_w, 1.0)
    return np.array([pos_loss, neg_loss, reg_loss], dtype=np.float32)

